# revision 22
# baseline (speedup 1.0000x reference)
"""CoaT serial block (ConvPosEnc + FactorAtt-ConvRelPosEnc + MLP) on 8 trn2
NeuronCores, data-parallel over batch (16 -> 2 per core).

v3 layout strategy (per core):
  - Master activations FEATURE-major: x^T [C=256 (2 part-tiles), N=3137] fp32;
    matmul operands bf16 (weights pre-concatenated + converted host-side,
    loaded in one DMA).
  - Depthwise convs (cpe 3x3 on x; crpe 3/5/7 on v) on the tensor engine as
    32x32 diagonal-block matmuls. Each image-row chunk is assigned one
    rotation class r=chunk%4 (positions (ii, (ii+r)%4)); ALL taps of a chunk
    accumulate into ONE PSUM bank, 4 chunks stream concurrently (16
    sub-arrays). Consumers read PSUM directly.
  - crpe output ev = q*(conv+bias) kept separate from factor; proj matmuls
    accumulate Wp.T@fac + Wp.T@ev in PSUM (no att tensor).
  - LayerNorm: per-token stats via N=1 matmuls (chunk-pipelined with the
    producer); rstd/-mu*rstd rows PE-transposed + DMA-gathered into a [2,*]
    row tile; the per-token broadcast AND d2 = g*negd + b are computed by
    tiny PE outer-product matmuls into PSUM (no gpsimd), chunk-fused with
    the consumer (qkv / MLP) so the PE never idles long.
  - softmax(k) skips the max; K=65 contraction on the last token tile.
  - MLP output written into x in place; per-chunk output DMAs; chunked x
    loads let the next batch item's input stream in under the MLP tail.
"""

import numpy as np

B, N, C, HEADS = 16, 3137, 256, 8
CH = C // HEADS
HW = 56
NPIX = HW * HW            # 3136
PW = 62                   # padded image width/height (pad=3)
GUARD = 4
PADIMG = PW * PW          # 3844
PADBUF = PADIMG + 2 * GUARD
P = 128
NTOK_T = 25
LAST_M = N - 24 * P       # 65
SCALE = CH ** -0.5
RPC = 7                   # image rows per conv chunk
NCHUNK = HW // RPC        # 8
CONV_N = RPC * PW         # 434
CPX = RPC * HW            # 392
SEQ_CHUNKS = [(i * 512, min(512, N - i * 512)) for i in range((N + 511) // 512)]
XSPLIT = 1 + 28 * HW      # row-aligned x-load split (1569)

_COMPILED = {}


# ---------------------------------------------------------------- host prep --

def _grid(K):
    p = K // 2
    return [((dy - p) * PW + (dx - p)) for dy in range(K) for dx in range(K)]


def _pack_diag(wtile, T):
    """[128, T] weights -> 4-class rotated block-diag pack [128, 4*T*32]."""
    base = np.zeros((P, T * 32), np.float32)
    for t in range(T):
        for jb in range(4):
            base[32 * jb:32 * jb + 32, 32 * t:32 * t + 32] = np.diag(
                wtile[32 * jb:32 * jb + 32, t])
    out = np.zeros((P, 4 * T * 32), np.float32)
    for r in range(4):
        for ii in range(4):
            jj = (ii + r) % 4
            out[32 * ii:32 * ii + 32, r * T * 32:(r + 1) * T * 32] \
                [:, :] = base[32 * jj:32 * jj + 32, :]
    return out


WCAT = [("wq0", 256), ("wq1", 256), ("wkv0", 512), ("wkv1", 512),
        ("wp0", 256), ("wp1", 256), ("wf10", 1024), ("wf11", 1024)] + \
       [(f"wf2{i}", 256) for i in range(8)] + \
       [("cpk0", 1152), ("cpk1", 1152), ("crA", 3200), ("crB", 6272),
        ("vbr0", 128), ("vbr1", 128)]
WCAT_COLS = sum(c for _, c in WCAT)          # 18176

BCAT = (["qb0", "qb1", "pb0", "pb1"] + [f"f1b{i}" for i in range(8)] +
        ["f2b0", "f2b1", "ln1_g0", "ln1_g1", "ln1_b0", "ln1_b1",
         "ln2_g0", "ln2_g1", "ln2_b0", "ln2_b1", "cpe_b0", "cpe_b1",
         "crpe_be0", "crpe_be1"])             # 26 cols


def _prep_consts(w):
    qkv_w, qkv_b = w["qkv_w"], w["qkv_b"]
    vb = qkv_b[512:768]
    crpe_b_cat = np.concatenate([w["crpe_b3"], w["crpe_b5"], w["crpe_b7"]])
    wsum = np.concatenate([
        w["crpe_w3"].reshape(64, -1).sum(1),
        w["crpe_w5"].reshape(96, -1).sum(1),
        w["crpe_w7"].reshape(96, -1).sum(1)])

    wqT = np.ascontiguousarray((qkv_w[0:256] * SCALE).T).reshape(2, P, 256)
    wkvT = np.ascontiguousarray(qkv_w[256:768].T).reshape(2, P, 512)
    projT = np.ascontiguousarray(w["proj_w"].T).reshape(2, P, 256)
    fc1T = np.ascontiguousarray(w["fc1_w"].T).reshape(2, P, 1024)
    fc2T = np.ascontiguousarray(w["fc2_w"].T).reshape(8, P, 256)
    vbr = np.stack([np.broadcast_to(vb[128 * g:128 * g + 128], (P, P))
                    for g in range(2)])

    cw = w["cpe_w"][:, 0]
    cpk = [_pack_diag(cw[ct * P:(ct + 1) * P].reshape(P, 9), 9)
           for ct in range(2)]
    w3 = w["crpe_w3"][:, 0] / SCALE
    w5 = w["crpe_w5"][:, 0] / SCALE
    w7 = w["crpe_w7"][:, 0] / SCALE
    wA = np.zeros((P, 25), np.float32)
    wA[0:64] = np.pad(w3, ((0, 0), (1, 1), (1, 1))).reshape(64, 25)
    wA[64:128] = w5[0:64].reshape(64, 25)
    wB = np.zeros((P, 49), np.float32)
    wB[0:32] = np.pad(w5[64:96], ((0, 0), (1, 1), (1, 1))).reshape(32, 49)
    wB[32:128] = w7.reshape(96, 49)

    parts = {"wq0": wqT[0], "wq1": wqT[1], "wkv0": wkvT[0], "wkv1": wkvT[1],
             "wp0": projT[0], "wp1": projT[1], "wf10": fc1T[0],
             "wf11": fc1T[1], "cpk0": cpk[0], "cpk1": cpk[1],
             "crA": _pack_diag(wA, 25), "crB": _pack_diag(wB, 49),
             "vbr0": vbr[0], "vbr1": vbr[1]}
    for i in range(8):
        parts[f"wf2{i}"] = fc2T[i]
    wcat = np.concatenate([parts[nm] for nm, _ in WCAT], axis=1)
    assert wcat.shape == (P, WCAT_COLS)

    bias = {"qb0": qkv_b[0:128] * SCALE, "qb1": qkv_b[128:256] * SCALE,
            "pb0": w["proj_b"][0:128], "pb1": w["proj_b"][128:256],
            "f2b0": w["fc2_b"][0:128], "f2b1": w["fc2_b"][128:256],
            "cpe_b0": w["cpe_b"][0:128], "cpe_b1": w["cpe_b"][128:256]}
    for i in range(8):
        bias[f"f1b{i}"] = w["fc1_b"][128 * i:128 * i + 128]
    for ln in (1, 2):
        for ct in range(2):
            bias[f"ln{ln}_g{ct}"] = w[f"ln{ln}_g"][128 * ct:128 * ct + 128]
            bias[f"ln{ln}_b{ct}"] = w[f"ln{ln}_b"][128 * ct:128 * ct + 128]
    be = (crpe_b_cat + vb * wsum) / SCALE
    bias["crpe_be0"], bias["crpe_be1"] = be[0:128], be[128:256]
    bcat = np.stack([bias[nm] for nm in BCAT], axis=1)
    assert bcat.shape == (P, 26)

    # gbT: [2, 512] rows [g; b] per (ln, ct) in 128-col blocks for the PE
    # rank-2 d2 matmul
    gbt = np.zeros((2, 512), np.float32)
    for ln in (1, 2):
        for ct in range(2):
            c0 = ((ln - 1) * 2 + ct) * 128
            gbt[0, c0:c0 + 128] = bias[f"ln{ln}_g{ct}"]
            gbt[1, c0:c0 + 128] = bias[f"ln{ln}_b{ct}"]
    return {"wcat": wcat, "bcat": bcat, "gbt": gbt}


# ------------------------------------------------------------------- device --

def build(n_batch=2):
    import concourse.tile as tile
    from concourse import bacc, mybir
    from concourse.masks import make_identity

    F = mybir.dt.float32
    BF = mybir.dt.bfloat16
    AL = mybir.AluOpType
    AF = mybir.ActivationFunctionType

    nc = bacc.Bacc(None, target_bir_lowering=False)

    d = {}
    d["xT"] = nc.dram_tensor("xT", (n_batch, 2, P, N), F, kind="ExternalInput")
    d["out"] = nc.dram_tensor("out", (n_batch, 2, P, N), F,
                              kind="ExternalOutput")
    d["wcat"] = nc.dram_tensor("wcat", (P, WCAT_COLS), BF,
                               kind="ExternalInput")
    d["bcat"] = nc.dram_tensor("bcat", (P, 26), F, kind="ExternalInput")
    d["gbt"] = nc.dram_tensor("gbt", (2, 512), BF, kind="ExternalInput")

    with tile.TileContext(nc) as tc:
        _emit(nc, tc, mybir, F, BF, AL, AF, make_identity, n_batch, d)
    nc.finalize()
    return nc


def _emit(nc, tc, mybir, F, BF, AL, AF, make_identity, n_batch, d):
    from contextlib import ExitStack
    with ExitStack() as ctx:
        wpool = ctx.enter_context(tc.tile_pool(name="wpool", bufs=1))
        mast = ctx.enter_context(tc.tile_pool(name="mast", bufs=1))
        work = ctx.enter_context(tc.tile_pool(name="work", bufs=1))
        cpool = ctx.enter_context(tc.tile_pool(name="cpool", bufs=1))
        ps = ctx.enter_context(tc.tile_pool(name="ps", bufs=1, space="PSUM"))

        wcat = wpool.tile([P, WCAT_COLS], BF, name="wcat", tag="wcat")
        nc.sync.dma_start(out=wcat, in_=d["wcat"][:, :])
        K = {}
        off = 0
        for nm, cols in WCAT:
            K[nm] = wcat[:, off:off + cols]
            off += cols
        bcat = wpool.tile([P, 26], F, name="bcat", tag="bcat")
        nc.sync.dma_start(out=bcat, in_=d["bcat"][:, :])
        pc = {nm: bcat[:, i:i + 1] for i, nm in enumerate(BCAT)}
        gbt = wpool.tile([2, 512], BF, name="gbt", tag="gbt")
        nc.sync.dma_start(out=gbt, in_=d["gbt"][:, :])

        ones_col = wpool.tile([P, 1], BF, name="ones_col", tag="ones_col")
        nc.vector.memset(ones_col, 1.0)
        ones_f32 = wpool.tile([P, 1], F, name="ones_f32", tag="ones_f32")
        nc.vector.memset(ones_f32, 1.0)
        ident = wpool.tile([P, P], F, name="ident", tag="ident")
        make_identity(nc, ident)
        eps_col = wpool.tile([P, 1], F, name="eps_col", tag="eps_col")
        nc.vector.memset(eps_col, 1e-6)

        # LN row tile: [0, 0:3200]=rstd row, [0, 3200:6400]=negd row (DMA'd
        # per LN), [0, 6400:6528]=ones (lhsT of the broadcast matmul),
        # [1, 3200:6400]=ones (2nd row of the d2 rank-2 rhs).
        rows2 = wpool.tile([2, 6528], BF, name="rows2", tag="rows2")
        nc.vector.memset(rows2[0:1, 6400:6528], 1.0)
        # both rows of [3200:6400) start as ones; row 0 is overwritten by the
        # per-LN negd DMA gather (single-partition DVE access is rejected)
        nc.vector.memset(rows2[0:2, 3200:6400], 1.0)

        pads = []
        for ct in range(2):
            t = mast.tile([P, PADBUF], BF, name=f"pad{ct}", tag=f"pad{ct}")
            nc.vector.memset(t, 0.0)
            pads.append(t)

        env = dict(nc=nc, mybir=mybir, F=F, BF=BF, AL=AL, AF=AF, K=K, pc=pc,
                   gbt=gbt, rows2=rows2, ones_col=ones_col,
                   ones_f32=ones_f32, ident=ident, eps_col=eps_col,
                   wpool=wpool, mast=mast, work=work, cpool=cpool, ps=ps,
                   d=d, pads=pads)
        for b in range(n_batch):
            _one_batch(env, b)


def _mm(env, out, lhsT, rhs, start, stop, tp=None):
    env["nc"].tensor.matmul(out, lhsT, rhs, start=start, stop=stop,
                            tile_position=tp, skip_group_check=True)


def _one_batch(env, b):
    import os
    STOP = int(os.environ.get("KSTOP", "99"))
    nc, F, BF, AL, AF = env["nc"], env["F"], env["BF"], env["AL"], env["AF"]
    K, pc = env["K"], env["pc"]
    mast, work, cpool, ps = env["mast"], env["work"], env["cpool"], env["ps"]
    d = env["d"]
    pads = env["pads"]

    def bail(bufs):
        for ct in range(2):
            nc.sync.dma_start(out=d["out"][b, ct], in_=bufs[ct][:, :N])
        return True

    # ---------------- load x feature-major (2 row-aligned pieces) ----------
    # [P, 3200]: cols N..3200 are never written (token-tile-24 stat matmuls
    # read them; the garbage stays confined to fake-token lanes)
    x = [mast.tile([P, 3200], F, name=f"x{ct}", tag=f"x{ct}")
         for ct in range(2)]
    for ct in range(2):
        for (n0, nn) in ((0, XSPLIT), (XSPLIT, N - XSPLIT)):
            nc.sync.dma_start(out=x[ct][:, n0:n0 + nn],
                              in_=d["xT"][b, ct][:, n0:n0 + nn])

    # ---------------- cpe: pad, rotate, conv, resid -----------------------
    rots = [None, None]
    for ct in range(2):
        for (r0, r1) in ((0, 28), (28, 56)):
            nc.vector.tensor_copy(
                _pv(pads[ct])[:, 3 + r0:3 + r1, 3:59],
                x[ct][:, 1 + r0 * HW:1 + r1 * HW].rearrange(
                    "p (r w) -> p r w", w=HW))
        rots[ct] = _mk_rots(env, pads[ct], ct)
    for ct in range(2):
        _dwconv_tile(env, rots[ct], K[f"cpk{ct}"], _grid(3), 9,
                     consumer=("cpe", x[ct], pc[f"cpe_b{ct}"]))

    if STOP <= 1:
        return bail(x)

    # ---------------- LN1 fused with qkv ----------------------------------
    cur = [work.tile([P, 3200], BF, name=f"cur{ct}", tag=f"cur{ct}")
           for ct in range(2)]
    q = [work.tile([P, N], BF, name=f"q{ct}", tag=f"q{ct}") for ct in range(2)]
    kex = work.tile([P, NTOK_T * 256], BF, name="kex", tag="kex")
    vtm = work.tile([P, NTOK_T * 256], BF, name="vtm", tag="vtm")

    def qkv_chunk(c, n0, nn):
        for ft in range(2):
            pt = ps.tile([P, 512], F, name="qps", tag="mmps", bufs=2)
            for kt in range(2):
                _mm(env, pt[:, :nn], K[f"wq{kt}"][:, 128 * ft:128 * ft + 128],
                    cur[kt][:, n0:n0 + nn], kt == 0, kt == 1)
            nc.scalar.activation(out=q[ft][:, n0:n0 + nn], in_=pt[:, :nn],
                                 func=AF.Identity, bias=pc[f"qb{ft}"],
                                 scale=1.0)
        for tt in range(n0 // P, min(NTOK_T, (n0 + nn + P - 1) // P)):
            m = P if tt < 24 else LAST_M
            pt = ps.tile([P, 512], F, name="kvps", tag="mmps", bufs=2)
            for kt in range(2):
                _mm(env, pt, cur[kt][:, P * tt:P * tt + P], K[f"wkv{kt}"],
                    kt == 0, kt == 1)
            nc.scalar.activation(out=kex[:m, 256 * tt:256 * tt + 256],
                                 in_=pt[:m, 0:256], func=AF.Exp)
            nc.scalar.activation(out=vtm[:m, 256 * tt:256 * tt + 256],
                                 in_=pt[:m, 256:512], func=AF.Copy)

    _layernorm(env, x, cur, 1, qkv_chunk)

    # ---------------- ksum, kv (K=65 contraction on last tile) -------------
    ksum_ps = ps.tile([P, 2], F, name="ksum_ps", tag="sps")
    for g in range(2):
        for tt in range(NTOK_T):
            m = P if tt < 24 else LAST_M
            _mm(env, ksum_ps[:, g:g + 1],
                kex[:m, 256 * tt + 128 * g:256 * tt + 128 * g + 128],
                env["ones_col"][:m], tt == 0, tt == 24)
    rk = work.tile([P, 2], F, name="rk", tag="rk")
    nc.vector.reciprocal(rk, ksum_ps)
    kv = [work.tile([P, P], BF, name=f"kv{g}", tag=f"kv{g}") for g in range(2)]
    for g in range(2):
        kvp = ps.tile([P, P], F, name=f"kvp{g}", tag="kvg")
        for tt in range(NTOK_T):
            m = P if tt < 24 else LAST_M
            _mm(env, kvp, kex[:m, 256 * tt + 128 * g:256 * tt + 128 * g + 128],
                vtm[:m, 256 * tt + 128 * g:256 * tt + 128 * g + 128],
                tt == 0, tt == 24)
        nc.vector.scalar_tensor_tensor(out=kv[g], in0=kvp,
                                       scalar=rk[:, g:g + 1],
                                       in1=K[f"vbr{g}"],
                                       op0=AL.mult, op1=AL.add)

    # ---------------- v^T -> padded images (both tiles first) --------------
    for ct in range(2):
        for ch in range(NCHUNK):
            pt = ps.tile([P, 512], F, name="vps", tag="mmps", bufs=2)
            for kt in range(2):
                _mm(env, pt[:, :CPX],
                    K[f"wkv{kt}"][:, 256 + 128 * ct:256 + 128 * ct + 128],
                    cur[kt][:, 1 + CPX * ch:1 + CPX * ch + CPX],
                    kt == 0, kt == 1)
            nc.vector.tensor_copy(
                _pv(pads[ct])[:, 3 + RPC * ch:3 + RPC * ch + RPC, 3:59],
                pt[:, :CPX].rearrange("p (r w) -> p r w", w=HW))
        rots[ct] = _mk_rots(env, pads[ct], ct)

    # ---------------- factor ----------------------------------------------
    fac = [work.tile([P, N], BF, name=f"fac{g}", tag=("kex", "vtm")[g])
           for g in range(2)]
    for g in range(2):
        for (n0, nn) in SEQ_CHUNKS:
            pt = ps.tile([P, 512], F, name="fps", tag="mmps", bufs=2)
            for hh in range(4):
                s = 32 * hh
                _mm(env, pt[s:s + 32, :nn], kv[g][s:s + 32, s:s + 32],
                    q[g][s:s + 32, n0:n0 + nn], True, True, tp=(s, s))
            nc.scalar.activation(out=fac[g][:, n0:n0 + nn], in_=pt[:, :nn],
                                 func=AF.Copy)

    # ---------------- crpe conv -> ev = q*(conv+bias) -----------------------
    ev = [work.tile([P, N], BF, name=f"ev{ct}", tag=f"cur{ct}")
          for ct in range(2)]
    for ct in range(2):
        nc.vector.memset(ev[ct][:, 0:1], 0.0)
    for ct in range(2):
        _dwconv_tile(env, rots[ct],
                     K["crA"] if ct == 0 else K["crB"],
                     _grid(5) if ct == 0 else _grid(7),
                     25 if ct == 0 else 49,
                     consumer=("crpe", (q[ct], ev[ct]), pc[f"crpe_be{ct}"]))

    if STOP <= 5:
        xs = [work.tile([P, N], F, name=f"dmp{ct}", tag=("kex", "vtm")[ct])
              for ct in range(2)]
        for ct in range(2):
            nc.vector.tensor_copy(xs[ct], ev[ct])
        return bail(xs)

    # ---------------- proj(fac + ev) + resid -> x (in place) ---------------
    for ft in range(2):
        for (n0, nn) in SEQ_CHUNKS:
            pt = ps.tile([P, 512], F, name="pps", tag="mmps", bufs=2)
            for kt in range(2):
                _mm(env, pt[:, :nn], K[f"wp{kt}"][:, 128 * ft:128 * ft + 128],
                    fac[kt][:, n0:n0 + nn], kt == 0, False)
            for kt in range(2):
                _mm(env, pt[:, :nn], K[f"wp{kt}"][:, 128 * ft:128 * ft + 128],
                    ev[kt][:, n0:n0 + nn], False, kt == 1)
            nc.vector.scalar_tensor_tensor(
                out=x[ft][:, n0:n0 + nn], in0=pt[:, :nn],
                scalar=pc[f"pb{ft}"], in1=x[ft][:, n0:n0 + nn],
                op0=AL.add, op1=AL.add)

    if STOP <= 6:
        return bail(x)

    # ---------------- LN2 fused with MLP; out streamed per chunk -----------
    cur2 = [work.tile([P, 3200], BF, name=f"cur2_{ct}", tag=f"cur{ct}")
            for ct in range(2)]

    def mlp_chunk(c, n0, nn):
        hb = []
        for ft in range(8):
            pt = ps.tile([P, 512], F, name="hps", tag="mmps", bufs=2)
            for kt in range(2):
                _mm(env, pt[:, :nn], K[f"wf1{kt}"][:, 128 * ft:128 * ft + 128],
                    cur2[kt][:, n0:n0 + nn], kt == 0, kt == 1)
            h = work.tile([P, 512], BF, name=f"h{ft}", tag=f"h{ft}")
            nc.scalar.activation(out=h[:, :nn], in_=pt[:, :nn], func=AF.Gelu,
                                 bias=pc[f"f1b{ft}"], scale=1.0)
            hb.append(h)
        for ct in range(2):
            pt2 = ps.tile([P, 512], F, name="ops", tag="mmps", bufs=2)
            for kt in range(8):
                _mm(env, pt2[:, :nn], K[f"wf2{kt}"][:, 128 * ct:128 * ct + 128],
                    hb[kt][:, :nn], kt == 0, kt == 7)
            nc.vector.scalar_tensor_tensor(
                out=x[ct][:, n0:n0 + nn], in0=pt2[:, :nn],
                scalar=pc[f"f2b{ct}"], in1=x[ct][:, n0:n0 + nn],
                op0=AL.add, op1=AL.add)
            nc.sync.dma_start(out=d["out"][b, ct][:, n0:n0 + nn],
                              in_=x[ct][:, n0:n0 + nn])

    _layernorm(env, x, cur2, 2, mlp_chunk)


def _pv(padt):
    return padt[:, GUARD:GUARD + PADIMG].rearrange("p (r w) -> p r w", w=PW)


def _mk_rots(env, pad, ct):
    """3 partition-rotated copies (2 DMAs each); A/B tag sets alternate by
    ct (rot3 shared: class-3 chunks are last per image)."""
    nc = env["nc"]
    ab = "AB"[ct]
    rots = [pad]
    for r in range(1, 4):
        tag = f"rot{ab}{r}" if r < 3 else "rot3"
        sr = env["cpool"].tile([P, PADBUF], env["BF"], name=tag, tag=tag)
        k = 32 * (4 - r)
        nc.sync.dma_start(out=sr[0:k], in_=pad[32 * r:128])
        nc.sync.dma_start(out=sr[k:128], in_=pad[0:32 * r])
        rots.append(sr)
    return rots


def _dwconv_tile(env, stacks, pack, offs, T, consumer):
    """Depthwise conv for one 128-channel image tile: chunk ch uses rotation
    class r=ch%4, one PSUM bank per chunk, 4 chunks concurrent."""
    nc, F, BF, AL = env["nc"], env["F"], env["BF"], env["AL"]
    ps, cpool = env["ps"], env["cpool"]
    kind = consumer[0]
    for g0 in range(0, NCHUNK, 4):
        chs = list(range(g0, min(g0 + 4, NCHUNK)))
        pts = {}
        for ch in chs:
            r = ch % 4
            pts[ch] = ps.tile([P, CONV_N], F, name=f"cv{r}", tag=f"cv{r}")
        for t in range(T):
            for ch in chs:
                r = ch % 4
                obase = GUARD + (3 + RPC * ch) * PW + offs[t]
                for ii in range(4):
                    jj = (ii + r) % 4
                    _mm(env, pts[ch][32 * jj:32 * jj + 32, :],
                        pack[32 * ii:32 * ii + 32,
                             (T * 32) * r + 32 * t:(T * 32) * r + 32 * t + 32],
                        stacks[r][32 * ii:32 * ii + 32,
                                  obase:obase + CONV_N],
                        t == 0, t == T - 1, tp=(32 * ii, 32 * jj))
        for ch in chs:
            sv = pts[ch].rearrange("p (r w) -> p r w", w=PW)[:, :, 3:59]
            px0 = CPX * ch
            if kind == "cpe":
                _, xm, bias = consumer
                xv = xm[:, 1 + px0:1 + px0 + CPX].rearrange(
                    "p (r w) -> p r w", w=HW)
                nc.vector.scalar_tensor_tensor(out=xv, in0=sv, scalar=bias,
                                               in1=xv, op0=AL.add, op1=AL.add)
            else:
                _, (qt, evt_), bias = consumer
                ev_v = evt_[:, 1 + px0:1 + px0 + CPX].rearrange(
                    "p (r w) -> p r w", w=HW)
                qv = qt[:, 1 + px0:1 + px0 + CPX].rearrange(
                    "p (r w) -> p r w", w=HW)
                nc.vector.scalar_tensor_tensor(out=ev_v, in0=sv, scalar=bias,
                                               in1=qv, op0=AL.add,
                                               op1=AL.mult)


def _layernorm(env, x, curo, ln, consume_chunk):
    """LN stats + chunk-fused apply: for each 512-token chunk, the rstd
    broadcast and d2 = g*negd + b are built by tiny PE matmuls into PSUM,
    cur chunk computed on DVE, then consume_chunk(c, n0, nn) emits the
    consumer's matmuls for that chunk."""
    nc, F, BF, AL, AF = env["nc"], env["F"], env["BF"], env["AL"], env["AF"]
    work, ps = env["work"], env["ps"]
    rows2, gbt = env["rows2"], env["gbt"]
    sq = []
    for ct in range(2):
        s = work.tile([P, 3200], BF, name=f"sq{ct}", tag=("kex", "vtm")[ct])
        for c, (n0, nn) in enumerate(SEQ_CHUNKS):
            if (c + ct) % 2 == 0:
                nc.scalar.activation(out=s[:, n0:n0 + nn],
                                     in_=x[ct][:, n0:n0 + nn],
                                     func=AF.Square)
            else:
                nc.vector.tensor_mul(s[:, n0:n0 + nn], x[ct][:, n0:n0 + nn],
                                     x[ct][:, n0:n0 + nn])
        sq.append(s)
    st = ps.tile([P, 64], F, name="lnstat", tag="sps")
    for tt in range(NTOK_T):
        for kt in range(2):
            _mm(env, st[:, 2 * tt:2 * tt + 1],
                x[kt][:, P * tt:P * tt + P], env["ones_f32"],
                kt == 0, kt == 1)
            _mm(env, st[:, 2 * tt + 1:2 * tt + 2],
                sq[kt][:, P * tt:P * tt + P], env["ones_col"],
                kt == 0, kt == 1)
    stv = st.rearrange("p (t two) -> p t two", two=2)
    mu = work.tile([P, NTOK_T], F, name="mu", tag="mu")
    nc.vector.tensor_scalar_mul(out=mu, in0=stv[:, 0:NTOK_T, 0],
                                scalar1=1.0 / C)
    var = work.tile([P, NTOK_T], F, name="var", tag="var")
    nc.vector.tensor_scalar_mul(out=var, in0=stv[:, 0:NTOK_T, 1],
                                scalar1=1.0 / C)
    mu2 = work.tile([P, NTOK_T], F, name="mu2", tag="mu2")
    nc.vector.tensor_mul(mu2, mu, mu)
    nc.vector.tensor_sub(var, var, mu2)
    nc.scalar.activation(out=var, in_=var, func=AF.Sqrt, bias=env["eps_col"],
                         scale=1.0)
    rstd = work.tile([P, NTOK_T], F, name="rstd", tag="rstd")
    nc.vector.reciprocal(rstd, var)
    negd = work.tile([P, NTOK_T], F, name="negd", tag="negd")
    nc.vector.tensor_mul(negd, mu, rstd)
    nc.vector.tensor_scalar_mul(out=negd, in0=negd, scalar1=-1.0)
    pk = work.tile([P, 64], F, name="lnpk", tag="lnpk")
    nc.vector.memset(pk, 0.0)
    nc.vector.tensor_copy(pk[:, 0:NTOK_T], rstd)
    nc.vector.tensor_copy(pk[:, 32:32 + NTOK_T], negd)
    tp = ps.tile([P, P], F, name="lntp", tag="kvg")
    nc.tensor.transpose(tp[0:64, :], pk, env["ident"])
    tps = work.tile([64, P], BF, name="lntps", tag="lntps")
    nc.vector.tensor_copy(tps, tp[0:64, :])
    nc.sync.dma_start(
        out=rows2[0:1, 0:3200].rearrange("o (t p) -> o t p", p=P),
        in_=tps[0:NTOK_T, :])
    nc.sync.dma_start(
        out=rows2[0:1, 3200:6400].rearrange("o (t p) -> o t p", p=P),
        in_=tps[32:32 + NTOK_T, :])

    for c, (n0, nn) in enumerate(SEQ_CHUNKS):
        rbp = ps.tile([P, 512], F, name="rbp", tag="cv0")
        _mm(env, rbp[:, :nn], rows2[0:1, 6400:6528],
            rows2[0:1, n0:n0 + nn], True, True)
        d2p = []
        for ct in range(2):
            dp = ps.tile([P, 512], F, name=f"d2p{ct}", tag=f"cv{1 + ct}")
            _mm(env, dp[:, :nn], gbt[:, ((ln - 1) * 2 + ct) * 128:
                                      ((ln - 1) * 2 + ct) * 128 + 128],
                rows2[0:2, 3200 + n0:3200 + n0 + nn], True, True)
            d2p.append(dp)
        for ct in range(2):
            nc.vector.scalar_tensor_tensor(
                out=curo[ct][:, n0:n0 + nn], in0=x[ct][:, n0:n0 + nn],
                scalar=pc_g(env, ln, ct), in1=rbp[:, :nn],
                op0=AL.mult, op1=AL.mult)
            nc.vector.tensor_add(curo[ct][:, n0:n0 + nn],
                                 curo[ct][:, n0:n0 + nn], d2p[ct][:, :nn])
        consume_chunk(c, n0, nn)


def pc_g(env, ln, ct):
    return env["pc"][f"ln{ln}_g{ct}"]


# --------------------------------------------------------------- entrypoint --

def kernel(**inputs):
    import ml_dtypes
    from concourse.bass_utils import run_bass_kernel_spmd

    x = np.asarray(inputs["x"], np.float32)
    assert int(inputs["H"]) == HW and int(inputs["W"]) == HW
    consts = _prep_consts({k: np.asarray(v, np.float32)
                           for k, v in inputs.items()
                           if k not in ("x", "H", "W")})

    if "main" not in _COMPILED:
        _COMPILED["main"] = build(n_batch=2)
    nc = _COMPILED["main"]

    n_cores = 8
    per = B // n_cores
    base = {
        "wcat": np.ascontiguousarray(consts["wcat"]).astype(
            ml_dtypes.bfloat16),
        "bcat": np.ascontiguousarray(consts["bcat"], np.float32),
        "gbt": np.ascontiguousarray(consts["gbt"]).astype(ml_dtypes.bfloat16),
    }

    in_maps = []
    for c in range(n_cores):
        xb = x[c * per:(c + 1) * per]
        m = dict(base)
        m["xT"] = np.ascontiguousarray(xb.transpose(0, 2, 1)).reshape(
            per, 2, P, N)
        in_maps.append(m)

    global _last_in_maps
    _last_in_maps = in_maps
    res = run_bass_kernel_spmd(nc, in_maps, list(range(n_cores))).results
    outs = [res[c]["out"].reshape(per, C, N).transpose(0, 2, 1)
            for c in range(n_cores)]
    return np.concatenate(outs, axis=0).astype(np.float32)


# revision 24
# speedup vs baseline: 1.0837x; 1.0837x over previous
"""CoaT serial block (ConvPosEnc + FactorAtt-ConvRelPosEnc + MLP) on 8 trn2
NeuronCores, data-parallel over batch (16 -> 2 per core).

v3 layout strategy (per core):
  - Master activations FEATURE-major: x^T [C=256 (2 part-tiles), N=3137] fp32;
    matmul operands bf16 (weights pre-concatenated + converted host-side,
    loaded in one DMA).
  - Depthwise convs (cpe 3x3 on x; crpe 3/5/7 on v) on the tensor engine as
    32x32 diagonal-block matmuls. Each image-row chunk is assigned one
    rotation class r=chunk%4 (positions (ii, (ii+r)%4)); ALL taps of a chunk
    accumulate into ONE PSUM bank, 4 chunks stream concurrently (16
    sub-arrays). Consumers read PSUM directly.
  - crpe output ev = q*(conv+bias) kept separate from factor; proj matmuls
    accumulate Wp.T@fac + Wp.T@ev in PSUM (no att tensor).
  - LayerNorm: per-token stats via N=1 matmuls (chunk-pipelined with the
    producer); rstd/-mu*rstd rows PE-transposed + DMA-gathered into a [2,*]
    row tile; the per-token broadcast AND d2 = g*negd + b are computed by
    tiny PE outer-product matmuls into PSUM (no gpsimd), chunk-fused with
    the consumer (qkv / MLP) so the PE never idles long.
  - softmax(k) skips the max; K=65 contraction on the last token tile.
  - MLP output written into x in place; per-chunk output DMAs; chunked x
    loads let the next batch item's input stream in under the MLP tail.
"""

import numpy as np

B, N, C, HEADS = 16, 3137, 256, 8
CH = C // HEADS
HW = 56
NPIX = HW * HW            # 3136
PW = 62                   # padded image width/height (pad=3)
GUARD = 4
PADIMG = PW * PW          # 3844
PADBUF = PADIMG + 2 * GUARD
P = 128
NTOK_T = 25
LAST_M = N - 24 * P       # 65
SCALE = CH ** -0.5
RPC = 7                   # image rows per conv chunk
NCHUNK = HW // RPC        # 8
CONV_N = RPC * PW         # 434
CPX = RPC * HW            # 392
SEQ_CHUNKS = [(i * 512, min(512, N - i * 512)) for i in range((N + 511) // 512)]
XSPLIT = 1 + 28 * HW      # row-aligned x-load split (1569)

_COMPILED = {}


# ---------------------------------------------------------------- host prep --

def _grid(K):
    p = K // 2
    return [((dy - p) * PW + (dx - p)) for dy in range(K) for dx in range(K)]


def _pack_diag(wtile, T):
    """[128, T] weights -> 4-class rotated block-diag pack [128, 4*T*32]."""
    base = np.zeros((P, T * 32), np.float32)
    for t in range(T):
        for jb in range(4):
            base[32 * jb:32 * jb + 32, 32 * t:32 * t + 32] = np.diag(
                wtile[32 * jb:32 * jb + 32, t])
    out = np.zeros((P, 4 * T * 32), np.float32)
    for r in range(4):
        for ii in range(4):
            jj = (ii + r) % 4
            out[32 * ii:32 * ii + 32, r * T * 32:(r + 1) * T * 32] \
                [:, :] = base[32 * jj:32 * jj + 32, :]
    return out


WCAT = [("wq0", 256), ("wq1", 256), ("wkv0", 512), ("wkv1", 512),
        ("wp0", 256), ("wp1", 256), ("wf10", 1024), ("wf11", 1024)] + \
       [(f"wf2{i}", 256) for i in range(8)] + \
       [("cpk0", 1152), ("cpk1", 1152), ("crA", 3200), ("crB", 6272),
        ("vbr0", 128), ("vbr1", 128)]
WCAT_COLS = sum(c for _, c in WCAT)          # 18176

BCAT = (["qb0", "qb1", "pb0", "pb1"] + [f"f1b{i}" for i in range(8)] +
        ["f2b0", "f2b1", "ln1_g0", "ln1_g1", "ln1_b0", "ln1_b1",
         "ln2_g0", "ln2_g1", "ln2_b0", "ln2_b1", "cpe_b0", "cpe_b1",
         "crpe_be0", "crpe_be1"])             # 26 cols


def _prep_consts(w):
    qkv_w, qkv_b = w["qkv_w"], w["qkv_b"]
    vb = qkv_b[512:768]
    crpe_b_cat = np.concatenate([w["crpe_b3"], w["crpe_b5"], w["crpe_b7"]])
    wsum = np.concatenate([
        w["crpe_w3"].reshape(64, -1).sum(1),
        w["crpe_w5"].reshape(96, -1).sum(1),
        w["crpe_w7"].reshape(96, -1).sum(1)])

    wqT = np.ascontiguousarray((qkv_w[0:256] * SCALE).T).reshape(2, P, 256)
    wkvT = np.ascontiguousarray(qkv_w[256:768].T).reshape(2, P, 512)
    projT = np.ascontiguousarray(w["proj_w"].T).reshape(2, P, 256)
    fc1T = np.ascontiguousarray(w["fc1_w"].T).reshape(2, P, 1024)
    fc2T = np.ascontiguousarray(w["fc2_w"].T).reshape(8, P, 256)
    vbr = np.stack([np.broadcast_to(vb[128 * g:128 * g + 128], (P, P))
                    for g in range(2)])

    cw = w["cpe_w"][:, 0]
    cpk = [_pack_diag(cw[ct * P:(ct + 1) * P].reshape(P, 9), 9)
           for ct in range(2)]
    w3 = w["crpe_w3"][:, 0] / SCALE
    w5 = w["crpe_w5"][:, 0] / SCALE
    w7 = w["crpe_w7"][:, 0] / SCALE
    wA = np.zeros((P, 25), np.float32)
    wA[0:64] = np.pad(w3, ((0, 0), (1, 1), (1, 1))).reshape(64, 25)
    wA[64:128] = w5[0:64].reshape(64, 25)
    wB = np.zeros((P, 49), np.float32)
    wB[0:32] = np.pad(w5[64:96], ((0, 0), (1, 1), (1, 1))).reshape(32, 49)
    wB[32:128] = w7.reshape(96, 49)

    parts = {"wq0": wqT[0], "wq1": wqT[1], "wkv0": wkvT[0], "wkv1": wkvT[1],
             "wp0": projT[0], "wp1": projT[1], "wf10": fc1T[0],
             "wf11": fc1T[1], "cpk0": cpk[0], "cpk1": cpk[1],
             "crA": _pack_diag(wA, 25), "crB": _pack_diag(wB, 49),
             "vbr0": vbr[0], "vbr1": vbr[1]}
    for i in range(8):
        parts[f"wf2{i}"] = fc2T[i]
    wcat = np.concatenate([parts[nm] for nm, _ in WCAT], axis=1)
    assert wcat.shape == (P, WCAT_COLS)

    bias = {"qb0": qkv_b[0:128] * SCALE, "qb1": qkv_b[128:256] * SCALE,
            "pb0": w["proj_b"][0:128], "pb1": w["proj_b"][128:256],
            "f2b0": w["fc2_b"][0:128], "f2b1": w["fc2_b"][128:256],
            "cpe_b0": w["cpe_b"][0:128], "cpe_b1": w["cpe_b"][128:256]}
    for i in range(8):
        bias[f"f1b{i}"] = w["fc1_b"][128 * i:128 * i + 128]
    for ln in (1, 2):
        for ct in range(2):
            bias[f"ln{ln}_g{ct}"] = w[f"ln{ln}_g"][128 * ct:128 * ct + 128]
            bias[f"ln{ln}_b{ct}"] = w[f"ln{ln}_b"][128 * ct:128 * ct + 128]
    be = (crpe_b_cat + vb * wsum) / SCALE
    bias["crpe_be0"], bias["crpe_be1"] = be[0:128], be[128:256]
    bcat = np.stack([bias[nm] for nm in BCAT], axis=1)
    assert bcat.shape == (P, 26)

    # gbT: [2, 512] rows [g; b] per (ln, ct) in 128-col blocks for the PE
    # rank-2 d2 matmul
    gbt = np.zeros((2, 512), np.float32)
    for ln in (1, 2):
        for ct in range(2):
            c0 = ((ln - 1) * 2 + ct) * 128
            gbt[0, c0:c0 + 128] = bias[f"ln{ln}_g{ct}"]
            gbt[1, c0:c0 + 128] = bias[f"ln{ln}_b{ct}"]
    return {"wcat": wcat, "bcat": bcat, "gbt": gbt}


# ------------------------------------------------------------------- device --

def build(n_batch=2):
    import concourse.tile as tile
    from concourse import bacc, mybir
    from concourse.masks import make_identity

    F = mybir.dt.float32
    BF = mybir.dt.bfloat16
    AL = mybir.AluOpType
    AF = mybir.ActivationFunctionType

    nc = bacc.Bacc(None, target_bir_lowering=False)

    d = {}
    d["xT"] = nc.dram_tensor("xT", (n_batch, 2, P, N), F, kind="ExternalInput")
    d["out"] = nc.dram_tensor("out", (n_batch, 2, P, N), F,
                              kind="ExternalOutput")
    d["wcat"] = nc.dram_tensor("wcat", (P, WCAT_COLS), BF,
                               kind="ExternalInput")
    d["bcat"] = nc.dram_tensor("bcat", (P, 26), F, kind="ExternalInput")
    d["gbt"] = nc.dram_tensor("gbt", (2, 512), BF, kind="ExternalInput")

    with tile.TileContext(nc) as tc:
        _emit(nc, tc, mybir, F, BF, AL, AF, make_identity, n_batch, d)
    nc.finalize()
    return nc


def _emit(nc, tc, mybir, F, BF, AL, AF, make_identity, n_batch, d):
    from contextlib import ExitStack
    with ExitStack() as ctx:
        wpool = ctx.enter_context(tc.tile_pool(name="wpool", bufs=1))
        mast = ctx.enter_context(tc.tile_pool(name="mast", bufs=1))
        work = ctx.enter_context(tc.tile_pool(name="work", bufs=1))
        cpool = ctx.enter_context(tc.tile_pool(name="cpool", bufs=1))
        ps = ctx.enter_context(tc.tile_pool(name="ps", bufs=1, space="PSUM"))

        wcat = wpool.tile([P, WCAT_COLS], BF, name="wcat", tag="wcat")
        nc.sync.dma_start(out=wcat, in_=d["wcat"][:, :])
        K = {}
        off = 0
        for nm, cols in WCAT:
            K[nm] = wcat[:, off:off + cols]
            off += cols
        bcat = wpool.tile([P, 26], F, name="bcat", tag="bcat")
        nc.sync.dma_start(out=bcat, in_=d["bcat"][:, :])
        pc = {nm: bcat[:, i:i + 1] for i, nm in enumerate(BCAT)}
        gbt = wpool.tile([2, 512], BF, name="gbt", tag="gbt")
        nc.sync.dma_start(out=gbt, in_=d["gbt"][:, :])

        ones_col = wpool.tile([P, 1], BF, name="ones_col", tag="ones_col")
        nc.vector.memset(ones_col, 1.0)
        ident = wpool.tile([P, P], F, name="ident", tag="ident")
        make_identity(nc, ident)
        eps_col = wpool.tile([P, 1], F, name="eps_col", tag="eps_col")
        nc.vector.memset(eps_col, 1e-6)

        # LN row tile: [0, 0:3200]=rstd row, [0, 3200:6400]=negd row (DMA'd
        # per LN), [0, 6400:6528]=ones (lhsT of the broadcast matmul),
        # [1, 3200:6400]=ones (2nd row of the d2 rank-2 rhs).
        rows2 = wpool.tile([2, 6528], BF, name="rows2", tag="rows2")
        nc.vector.memset(rows2[0:1, 6400:6528], 1.0)
        # both rows of [3200:6400) start as ones; row 0 is overwritten by the
        # per-LN negd DMA gather (single-partition DVE access is rejected)
        nc.vector.memset(rows2[0:2, 3200:6400], 1.0)

        pads = []
        for ct in range(2):
            t = mast.tile([P, PADBUF], BF, name=f"pad{ct}", tag=f"pad{ct}")
            nc.vector.memset(t, 0.0)
            pads.append(t)

        env = dict(nc=nc, mybir=mybir, F=F, BF=BF, AL=AL, AF=AF, K=K, pc=pc,
                   gbt=gbt, rows2=rows2, ones_col=ones_col, ident=ident,
                   eps_col=eps_col, wpool=wpool, mast=mast, work=work,
                   cpool=cpool, ps=ps, d=d, pads=pads)
        for b in range(n_batch):
            _one_batch(env, b)


def _mm(env, out, lhsT, rhs, start, stop, tp=None):
    env["nc"].tensor.matmul(out, lhsT, rhs, start=start, stop=stop,
                            tile_position=tp, skip_group_check=True)


def _one_batch(env, b):
    import os
    STOP = int(os.environ.get("KSTOP", "99"))
    nc, F, BF, AL, AF = env["nc"], env["F"], env["BF"], env["AL"], env["AF"]
    K, pc = env["K"], env["pc"]
    mast, work, cpool, ps = env["mast"], env["work"], env["cpool"], env["ps"]
    d = env["d"]
    pads = env["pads"]

    def bail(bufs):
        for ct in range(2):
            nc.sync.dma_start(out=d["out"][b, ct], in_=bufs[ct])
        return True

    # ---------------- load x feature-major (2 row-aligned pieces) ----------
    x = [mast.tile([P, N], F, name=f"x{ct}", tag=f"x{ct}") for ct in range(2)]
    for ct in range(2):
        for (n0, nn) in ((0, XSPLIT), (XSPLIT, N - XSPLIT)):
            nc.sync.dma_start(out=x[ct][:, n0:n0 + nn],
                              in_=d["xT"][b, ct][:, n0:n0 + nn])

    # ---------------- cpe: pad, rotate, conv, resid -----------------------
    rots = [None, None]
    for ct in range(2):
        for (r0, r1) in ((0, 28), (28, 56)):
            nc.vector.tensor_copy(
                _pv(pads[ct])[:, 3 + r0:3 + r1, 3:59],
                x[ct][:, 1 + r0 * HW:1 + r1 * HW].rearrange(
                    "p (r w) -> p r w", w=HW))
        rots[ct] = _mk_rots(env, pads[ct], ct)
    for ct in range(2):
        _dwconv_tile(env, rots[ct], K[f"cpk{ct}"], _grid(3), 9,
                     consumer=("cpe", x[ct], pc[f"cpe_b{ct}"]))

    if STOP <= 1:
        return bail(x)

    # ---------------- LN1 fused with qkv ----------------------------------
    cur = [work.tile([P, 3200], BF, name=f"cur{ct}", tag=f"cur{ct}")
           for ct in range(2)]
    q = [work.tile([P, N], BF, name=f"q{ct}", tag=f"q{ct}") for ct in range(2)]
    kex = work.tile([P, NTOK_T * 256], BF, name="kex", tag="kex")
    vtm = work.tile([P, NTOK_T * 256], BF, name="vtm", tag="vtm")

    def qkv_chunk(c, n0, nn):
        for ft in range(2):
            pt = ps.tile([P, 512], F, name="qps", tag="mmps", bufs=2)
            for kt in range(2):
                _mm(env, pt[:, :nn], K[f"wq{kt}"][:, 128 * ft:128 * ft + 128],
                    cur[kt][:, n0:n0 + nn], kt == 0, kt == 1)
            nc.scalar.activation(out=q[ft][:, n0:n0 + nn], in_=pt[:, :nn],
                                 func=AF.Identity, bias=pc[f"qb{ft}"],
                                 scale=1.0)
        for tt in range(n0 // P, min(NTOK_T, (n0 + nn + P - 1) // P)):
            m = P if tt < 24 else LAST_M
            pt = ps.tile([P, 512], F, name="kvps", tag="mmps", bufs=2)
            for kt in range(2):
                _mm(env, pt, cur[kt][:, P * tt:P * tt + P], K[f"wkv{kt}"],
                    kt == 0, kt == 1)
            nc.scalar.activation(out=kex[:m, 256 * tt:256 * tt + 256],
                                 in_=pt[:m, 0:256], func=AF.Exp)
            nc.vector.tensor_copy(vtm[:m, 256 * tt:256 * tt + 256],
                                  pt[:m, 256:512])

    _layernorm(env, x, cur, 1, qkv_chunk)

    # ---------------- ksum, kv (K=65 contraction on last tile) -------------
    ksum_ps = ps.tile([P, 2], F, name="ksum_ps", tag="sps")
    for g in range(2):
        for tt in range(NTOK_T):
            m = P if tt < 24 else LAST_M
            _mm(env, ksum_ps[:, g:g + 1],
                kex[:m, 256 * tt + 128 * g:256 * tt + 128 * g + 128],
                env["ones_col"][:m], tt == 0, tt == 24)
    rk = work.tile([P, 2], F, name="rk", tag="rk")
    nc.vector.reciprocal(rk, ksum_ps)
    kv = [work.tile([P, P], BF, name=f"kv{g}", tag=f"kv{g}") for g in range(2)]
    for g in range(2):
        kvp = ps.tile([P, P], F, name=f"kvp{g}", tag="kvg")
        for tt in range(NTOK_T):
            m = P if tt < 24 else LAST_M
            _mm(env, kvp, kex[:m, 256 * tt + 128 * g:256 * tt + 128 * g + 128],
                vtm[:m, 256 * tt + 128 * g:256 * tt + 128 * g + 128],
                tt == 0, tt == 24)
        nc.vector.scalar_tensor_tensor(out=kv[g], in0=kvp,
                                       scalar=rk[:, g:g + 1],
                                       in1=K[f"vbr{g}"],
                                       op0=AL.mult, op1=AL.add)

    # ---------------- v^T -> padded images (both tiles first) --------------
    for ct in range(2):
        for ch in range(NCHUNK):
            pt = ps.tile([P, 512], F, name="vps", tag="mmps", bufs=2)
            for kt in range(2):
                _mm(env, pt[:, :CPX],
                    K[f"wkv{kt}"][:, 256 + 128 * ct:256 + 128 * ct + 128],
                    cur[kt][:, 1 + CPX * ch:1 + CPX * ch + CPX],
                    kt == 0, kt == 1)
            nc.vector.tensor_copy(
                _pv(pads[ct])[:, 3 + RPC * ch:3 + RPC * ch + RPC, 3:59],
                pt[:, :CPX].rearrange("p (r w) -> p r w", w=HW))
        rots[ct] = _mk_rots(env, pads[ct], ct)

    # ---------------- factor ----------------------------------------------
    fac = [work.tile([P, N], BF, name=f"fac{g}", tag=("kex", "vtm")[g])
           for g in range(2)]
    for g in range(2):
        for (n0, nn) in SEQ_CHUNKS:
            pt = ps.tile([P, 512], F, name="fps", tag="mmps", bufs=2)
            for hh in range(4):
                s = 32 * hh
                _mm(env, pt[s:s + 32, :nn], kv[g][s:s + 32, s:s + 32],
                    q[g][s:s + 32, n0:n0 + nn], True, True, tp=(s, s))
            nc.vector.tensor_copy(fac[g][:, n0:n0 + nn], pt[:, :nn])

    # ---------------- crpe conv -> ev = q*(conv+bias) -----------------------
    ev = [work.tile([P, N], BF, name=f"ev{ct}", tag=f"cur{ct}")
          for ct in range(2)]
    for ct in range(2):
        nc.vector.memset(ev[ct][:, 0:1], 0.0)
    for ct in range(2):
        _dwconv_tile(env, rots[ct],
                     K["crA"] if ct == 0 else K["crB"],
                     _grid(5) if ct == 0 else _grid(7),
                     25 if ct == 0 else 49,
                     consumer=("crpe", (q[ct], ev[ct]), pc[f"crpe_be{ct}"]))

    if STOP <= 5:
        xs = [work.tile([P, N], F, name=f"dmp{ct}", tag=("kex", "vtm")[ct])
              for ct in range(2)]
        for ct in range(2):
            nc.vector.tensor_copy(xs[ct], ev[ct])
        return bail(xs)

    # ---------------- proj(fac + ev) + resid -> x (in place) ---------------
    for ft in range(2):
        for (n0, nn) in SEQ_CHUNKS:
            pt = ps.tile([P, 512], F, name="pps", tag="mmps", bufs=2)
            for kt in range(2):
                _mm(env, pt[:, :nn], K[f"wp{kt}"][:, 128 * ft:128 * ft + 128],
                    fac[kt][:, n0:n0 + nn], kt == 0, False)
            for kt in range(2):
                _mm(env, pt[:, :nn], K[f"wp{kt}"][:, 128 * ft:128 * ft + 128],
                    ev[kt][:, n0:n0 + nn], False, kt == 1)
            nc.vector.scalar_tensor_tensor(
                out=x[ft][:, n0:n0 + nn], in0=pt[:, :nn],
                scalar=pc[f"pb{ft}"], in1=x[ft][:, n0:n0 + nn],
                op0=AL.add, op1=AL.add)

    if STOP <= 6:
        return bail(x)

    # ---------------- LN2 fused with MLP; out streamed per chunk -----------
    cur2 = [work.tile([P, 3200], BF, name=f"cur2_{ct}", tag=f"cur{ct}")
            for ct in range(2)]

    def mlp_chunk(c, n0, nn):
        hb = []
        for ft in range(8):
            pt = ps.tile([P, 512], F, name="hps", tag="mmps", bufs=2)
            for kt in range(2):
                _mm(env, pt[:, :nn], K[f"wf1{kt}"][:, 128 * ft:128 * ft + 128],
                    cur2[kt][:, n0:n0 + nn], kt == 0, kt == 1)
            h = work.tile([P, 512], BF, name=f"h{ft}", tag=f"h{ft}")
            nc.scalar.activation(out=h[:, :nn], in_=pt[:, :nn], func=AF.Gelu,
                                 bias=pc[f"f1b{ft}"], scale=1.0)
            hb.append(h)
        for ct in range(2):
            pt2 = ps.tile([P, 512], F, name="ops", tag="mmps", bufs=2)
            for kt in range(8):
                _mm(env, pt2[:, :nn], K[f"wf2{kt}"][:, 128 * ct:128 * ct + 128],
                    hb[kt][:, :nn], kt == 0, kt == 7)
            nc.vector.scalar_tensor_tensor(
                out=x[ct][:, n0:n0 + nn], in0=pt2[:, :nn],
                scalar=pc[f"f2b{ct}"], in1=x[ct][:, n0:n0 + nn],
                op0=AL.add, op1=AL.add)
            nc.sync.dma_start(out=d["out"][b, ct][:, n0:n0 + nn],
                              in_=x[ct][:, n0:n0 + nn])

    _layernorm(env, x, cur2, 2, mlp_chunk)


def _pv(padt):
    return padt[:, GUARD:GUARD + PADIMG].rearrange("p (r w) -> p r w", w=PW)


def _mk_rots(env, pad, ct):
    """3 partition-rotated copies (2 DMAs each); A/B tag sets alternate by
    ct (rot3 shared: class-3 chunks are last per image)."""
    nc = env["nc"]
    ab = "AB"[ct]
    rots = [pad]
    for r in range(1, 4):
        tag = f"rot{ab}{r}" if r < 3 else "rot3"
        sr = env["cpool"].tile([P, PADBUF], env["BF"], name=tag, tag=tag)
        k = 32 * (4 - r)
        nc.sync.dma_start(out=sr[0:k], in_=pad[32 * r:128])
        nc.sync.dma_start(out=sr[k:128], in_=pad[0:32 * r])
        rots.append(sr)
    return rots


def _dwconv_tile(env, stacks, pack, offs, T, consumer):
    """Depthwise conv for one 128-channel image tile: chunk ch uses rotation
    class r=ch%4, one PSUM bank per chunk, 4 chunks concurrent."""
    nc, F, BF, AL = env["nc"], env["F"], env["BF"], env["AL"]
    ps, cpool = env["ps"], env["cpool"]
    kind = consumer[0]
    for g0 in range(0, NCHUNK, 4):
        chs = list(range(g0, min(g0 + 4, NCHUNK)))
        pts = {}
        for ch in chs:
            r = ch % 4
            pts[ch] = ps.tile([P, CONV_N], F, name=f"cv{r}", tag=f"cv{r}")
        for t in range(T):
            for ch in chs:
                r = ch % 4
                obase = GUARD + (3 + RPC * ch) * PW + offs[t]
                for ii in range(4):
                    jj = (ii + r) % 4
                    _mm(env, pts[ch][32 * jj:32 * jj + 32, :],
                        pack[32 * ii:32 * ii + 32,
                             (T * 32) * r + 32 * t:(T * 32) * r + 32 * t + 32],
                        stacks[r][32 * ii:32 * ii + 32,
                                  obase:obase + CONV_N],
                        t == 0, t == T - 1, tp=(32 * ii, 32 * jj))
        for ch in chs:
            sv = pts[ch].rearrange("p (r w) -> p r w", w=PW)[:, :, 3:59]
            px0 = CPX * ch
            if kind == "cpe":
                _, xm, bias = consumer
                xv = xm[:, 1 + px0:1 + px0 + CPX].rearrange(
                    "p (r w) -> p r w", w=HW)
                nc.vector.scalar_tensor_tensor(out=xv, in0=sv, scalar=bias,
                                               in1=xv, op0=AL.add, op1=AL.add)
            else:
                _, (qt, evt_), bias = consumer
                tmp = cpool.tile([P, CPX], BF, name="evt", tag="evt", bufs=2)
                nc.vector.tensor_scalar_add(
                    out=tmp.rearrange("p (r w) -> p r w", w=HW), in0=sv,
                    scalar1=bias)
                nc.vector.tensor_mul(evt_[:, 1 + px0:1 + px0 + CPX], tmp,
                                     qt[:, 1 + px0:1 + px0 + CPX])


def _layernorm(env, x, curo, ln, consume_chunk):
    """LN stats + chunk-fused apply: for each 512-token chunk, the rstd
    broadcast and d2 = g*negd + b are built by tiny PE matmuls into PSUM,
    cur chunk computed on DVE, then consume_chunk(c, n0, nn) emits the
    consumer's matmuls for that chunk."""
    nc, F, BF, AL, AF = env["nc"], env["F"], env["BF"], env["AL"], env["AF"]
    work, ps = env["work"], env["ps"]
    rows2, gbt = env["rows2"], env["gbt"]
    xbf = []
    sq = []
    for ct in range(2):
        xb = work.tile([P, 3200], BF, name=f"xbf{ct}", tag=f"xbf{ct}")
        s = work.tile([P, 3200], BF, name=f"sq{ct}", tag=("kex", "vtm")[ct])
        for (n0, nn) in SEQ_CHUNKS:
            nc.vector.tensor_copy(xb[:, n0:n0 + nn], x[ct][:, n0:n0 + nn])
            nc.scalar.activation(out=s[:, n0:n0 + nn], in_=xb[:, n0:n0 + nn],
                                 func=AF.Square)
        xbf.append(xb)
        sq.append(s)
    st = ps.tile([P, 64], F, name="lnstat", tag="sps")
    for tt in range(NTOK_T):
        for kt in range(2):
            _mm(env, st[:, 2 * tt:2 * tt + 1],
                xbf[kt][:, P * tt:P * tt + P], env["ones_col"],
                kt == 0, kt == 1)
            _mm(env, st[:, 2 * tt + 1:2 * tt + 2],
                sq[kt][:, P * tt:P * tt + P], env["ones_col"],
                kt == 0, kt == 1)
    stv = st.rearrange("p (t two) -> p t two", two=2)
    mu = work.tile([P, NTOK_T], F, name="mu", tag="mu")
    nc.vector.tensor_scalar_mul(out=mu, in0=stv[:, 0:NTOK_T, 0],
                                scalar1=1.0 / C)
    var = work.tile([P, NTOK_T], F, name="var", tag="var")
    nc.vector.tensor_scalar_mul(out=var, in0=stv[:, 0:NTOK_T, 1],
                                scalar1=1.0 / C)
    mu2 = work.tile([P, NTOK_T], F, name="mu2", tag="mu2")
    nc.vector.tensor_mul(mu2, mu, mu)
    nc.vector.tensor_sub(var, var, mu2)
    nc.scalar.activation(out=var, in_=var, func=AF.Ln, bias=env["eps_col"],
                         scale=1.0)
    rstd = work.tile([P, NTOK_T], F, name="rstd", tag="rstd")
    nc.scalar.activation(out=rstd, in_=var, func=AF.Exp, bias=0.0, scale=-0.5)
    negd = work.tile([P, NTOK_T], F, name="negd", tag="negd")
    nc.vector.tensor_mul(negd, mu, rstd)
    nc.vector.tensor_scalar_mul(out=negd, in0=negd, scalar1=-1.0)
    pk = work.tile([P, 64], F, name="lnpk", tag="lnpk")
    nc.vector.memset(pk, 0.0)
    nc.vector.tensor_copy(pk[:, 0:NTOK_T], rstd)
    nc.vector.tensor_copy(pk[:, 32:32 + NTOK_T], negd)
    tp = ps.tile([P, P], F, name="lntp", tag="kvg")
    nc.tensor.transpose(tp[0:64, :], pk, env["ident"])
    tps = work.tile([64, P], BF, name="lntps", tag="lntps")
    nc.vector.tensor_copy(tps, tp[0:64, :])
    nc.sync.dma_start(
        out=rows2[0:1, 0:3200].rearrange("o (t p) -> o t p", p=P),
        in_=tps[0:NTOK_T, :])
    nc.sync.dma_start(
        out=rows2[0:1, 3200:6400].rearrange("o (t p) -> o t p", p=P),
        in_=tps[32:32 + NTOK_T, :])

    for c, (n0, nn) in enumerate(SEQ_CHUNKS):
        rbp = ps.tile([P, 512], F, name="rbp", tag="cv0")
        _mm(env, rbp[:, :nn], rows2[0:1, 6400:6528],
            rows2[0:1, n0:n0 + nn], True, True)
        d2p = []
        for ct in range(2):
            dp = ps.tile([P, 512], F, name=f"d2p{ct}", tag=f"cv{1 + ct}")
            _mm(env, dp[:, :nn], gbt[:, ((ln - 1) * 2 + ct) * 128:
                                      ((ln - 1) * 2 + ct) * 128 + 128],
                rows2[0:2, 3200 + n0:3200 + n0 + nn], True, True)
            d2p.append(dp)
        for ct in range(2):
            nc.vector.scalar_tensor_tensor(
                out=curo[ct][:, n0:n0 + nn], in0=xbf[ct][:, n0:n0 + nn],
                scalar=pc_g(env, ln, ct), in1=rbp[:, :nn],
                op0=AL.mult, op1=AL.mult)
            nc.vector.tensor_add(curo[ct][:, n0:n0 + nn],
                                 curo[ct][:, n0:n0 + nn], d2p[ct][:, :nn])
        consume_chunk(c, n0, nn)


def pc_g(env, ln, ct):
    return env["pc"][f"ln{ln}_g{ct}"]


# --------------------------------------------------------------- entrypoint --

def kernel(**inputs):
    import ml_dtypes
    from concourse.bass_utils import run_bass_kernel_spmd

    x = np.asarray(inputs["x"], np.float32)
    assert int(inputs["H"]) == HW and int(inputs["W"]) == HW
    consts = _prep_consts({k: np.asarray(v, np.float32)
                           for k, v in inputs.items()
                           if k not in ("x", "H", "W")})

    if "main" not in _COMPILED:
        _COMPILED["main"] = build(n_batch=2)
    nc = _COMPILED["main"]

    n_cores = 8
    per = B // n_cores
    base = {
        "wcat": np.ascontiguousarray(consts["wcat"]).astype(
            ml_dtypes.bfloat16),
        "bcat": np.ascontiguousarray(consts["bcat"], np.float32),
        "gbt": np.ascontiguousarray(consts["gbt"]).astype(ml_dtypes.bfloat16),
    }

    in_maps = []
    for c in range(n_cores):
        xb = x[c * per:(c + 1) * per]
        m = dict(base)
        m["xT"] = np.ascontiguousarray(xb.transpose(0, 2, 1)).reshape(
            per, 2, P, N)
        in_maps.append(m)

    global _last_in_maps
    _last_in_maps = in_maps
    res = run_bass_kernel_spmd(nc, in_maps, list(range(n_cores))).results
    outs = [res[c]["out"].reshape(per, C, N).transpose(0, 2, 1)
            for c in range(n_cores)]
    return np.concatenate(outs, axis=0).astype(np.float32)
